# revision 1
# baseline (speedup 1.0000x reference)
"""Cross-graph attention (block-diagonal segment-local attention) on 8 trn2 cores.

Strategy: graphs (batch ids) are contiguous segments in the sorted
atom_batch / residue_batch arrays.  Attention is block-diagonal: atoms of
graph b attend only to residues of graph b.  We shard 4 graphs per core,
pad every graph to a fixed (AG atoms, RG residues) slot so all 8 cores run
one identical SPMD program, and compute per-graph attention with no masks:

  - inputs are packed host-side as transposed tiles atom_h^T (128, A_pad),
    residue_h^T (128, R_pad); zero padding makes padded K columns / V rows
    exactly 0.
  - scores are computed transposed,  S^T = K @ Q^T,  so every matmul takes
    naturally-laid-out operands (no on-device transposes anywhere).
  - exp(S/sqrt(128)) is one ACT instruction per tile (scale folded in).
  - V is augmented with a {0,1} "valid residue" column; the attention
    matmul U = expS^T.T @ [V | valid] then yields both the unnormalized
    context and the softmax denominator.  Padded residues have exp=1 but
    V row = 0 and valid = 0, so they contribute nothing.
  - normalization + residual add run host-side: out = atom_h + U[:, :128]/U[:, 128:].
"""

import sys

if "/opt/trn_rl_repo" not in sys.path:
    sys.path.insert(0, "/opt/trn_rl_repo")

import numpy as np

import concourse.bass as bass
import concourse.tile as tile
from concourse import bacc, mybir
from concourse.bass_utils import run_bass_kernel_spmd

N_CORES = 8
B = 32                      # number of graphs
P = 128                     # partitions
DH = 128                    # feature dims (DA == DR == DH == 128)
SCALE = 1.0 / np.sqrt(128.0)

_kernel_cache: dict = {}


def _build_kernel(AG: int, RG: int, G: int):
    """One SPMD program: G graph slots of (AG atoms, RG residues) per core."""
    A_pad = G * AG
    R_pad = G * RG
    nkg = RG // P               # residue chunks per graph
    nRc = G * nkg               # residue chunks per core
    ntg = AG // P               # atom chunks per graph
    nAc = G * ntg               # atom chunks per core
    f32 = mybir.dt.float32

    nc = bacc.Bacc("TRN2")
    atomT = nc.dram_tensor("atomT", [P, A_pad], f32, kind="ExternalInput")
    resT = nc.dram_tensor("resT", [P, R_pad], f32, kind="ExternalInput")
    wqT = nc.dram_tensor("wqT", [P, DH], f32, kind="ExternalInput")
    wkT = nc.dram_tensor("wkT", [P, DH], f32, kind="ExternalInput")
    wvT = nc.dram_tensor("wvT", [P, DH], f32, kind="ExternalInput")
    valid = nc.dram_tensor("valid", [P, nRc], f32, kind="ExternalInput")
    out = nc.dram_tensor("out", [A_pad, DH + 1], f32, kind="ExternalOutput")

    with tile.TileContext(nc) as tc:
        with (
            tc.tile_pool(name="singles", bufs=1) as singles,
            tc.tile_pool(name="psum_big", bufs=2, space="PSUM") as ps_big,
            tc.tile_pool(name="psum_small", bufs=2, space="PSUM") as ps_small,
        ):
            # ---- load everything to SBUF ----
            atomT_sb = singles.tile([P, A_pad], f32)
            resT_sb = singles.tile([P, R_pad], f32)
            wqT_sb = singles.tile([P, DH], f32)
            wkT_sb = singles.tile([P, DH], f32)
            wvT_sb = singles.tile([P, DH], f32)
            nc.sync.dma_start(atomT_sb[:], atomT[:])
            nc.sync.dma_start(resT_sb[:], resT[:])
            nc.sync.dma_start(wqT_sb[:], wqT[:])
            nc.sync.dma_start(wkT_sb[:], wkT[:])
            nc.sync.dma_start(wvT_sb[:], wvT[:])

            # V' = [residue_h @ W_v^T | valid] laid out per residue chunk
            V_sb = singles.tile([P, nRc, DH + 1], f32)
            nc.sync.dma_start(V_sb[:, :, DH], valid[:])

            # ---- Q^T = W_q @ atom_h^T, K^T = W_k @ residue_h^T ----
            QT_sb = singles.tile([P, A_pad], f32)
            for i in range(0, A_pad, 512):
                w = min(512, A_pad - i)
                pq = ps_big.tile([P, 512], f32, tag="big")
                nc.tensor.matmul(
                    pq[:, :w], wqT_sb[:], atomT_sb[:, i : i + w],
                    start=True, stop=True,
                )
                nc.any.tensor_copy(QT_sb[:, i : i + w], pq[:, :w])

            KT_sb = singles.tile([P, R_pad], f32)
            for i in range(0, R_pad, 512):
                w = min(512, R_pad - i)
                pk = ps_big.tile([P, 512], f32, tag="big")
                nc.tensor.matmul(
                    pk[:, :w], wkT_sb[:], resT_sb[:, i : i + w],
                    start=True, stop=True,
                )
                nc.any.tensor_copy(KT_sb[:, i : i + w], pk[:, :w])

            # ---- V chunks ----
            for k in range(nRc):
                pv = ps_small.tile([P, DH + 1], f32, tag="small")
                nc.tensor.matmul(
                    pv[:, :DH], resT_sb[:, k * P : (k + 1) * P], wvT_sb[:],
                    start=True, stop=True,
                )
                nc.any.tensor_copy(V_sb[:, k, :DH], pv[:, :DH])

            # ---- per-graph attention ----
            ES_sb = singles.tile([P, nRc, AG], f32)   # exp(S^T) per residue chunk
            OUT_sb = singles.tile([P, nAc, DH + 1], f32)

            for g in range(G):
                a0 = g * AG
                for k in range(nkg):
                    kg = g * nkg + k
                    r0 = kg * P
                    ps = ps_big.tile([P, 512 * ((AG + 511) // 512)], f32, tag="big")
                    for c in range(0, AG, 512):
                        w = min(512, AG - c)
                        nc.tensor.matmul(
                            ps[:, c : c + w],
                            KT_sb[:, r0 : r0 + P],
                            QT_sb[:, a0 + c : a0 + c + w],
                            start=True, stop=True,
                        )
                    nc.scalar.activation(
                        ES_sb[:, kg, :], ps[:, :AG],
                        mybir.ActivationFunctionType.Exp, scale=SCALE,
                    )

                for t in range(ntg):
                    tg = g * ntg + t
                    pu = ps_small.tile([P, DH + 1], f32, tag="small")
                    for k in range(nkg):
                        kg = g * nkg + k
                        nc.tensor.matmul(
                            pu[:],
                            ES_sb[:, kg, t * P : (t + 1) * P],
                            V_sb[:, kg, :],
                            start=(k == 0), stop=(k == nkg - 1),
                        )
                    nc.any.tensor_copy(OUT_sb[:, tg, :], pu[:])

            nc.sync.dma_start(
                out.rearrange("(t p) f -> p t f", p=P), OUT_sb[:]
            )

    nc.compile()
    return nc


def kernel(atom_h, residue_h, atom_batch, residue_batch, W_q, W_k, W_v):
    atom_h = np.asarray(atom_h, dtype=np.float32)
    residue_h = np.asarray(residue_h, dtype=np.float32)
    atom_batch = np.asarray(atom_batch)
    residue_batch = np.asarray(residue_batch)
    W_q = np.asarray(W_q, dtype=np.float32)
    W_k = np.asarray(W_k, dtype=np.float32)
    W_v = np.asarray(W_v, dtype=np.float32)

    A = atom_h.shape[0]
    R = residue_h.shape[0]
    n_b = max(B, int(atom_batch.max()) + 1 if A else B,
              int(residue_batch.max()) + 1 if R else B)

    ac = np.bincount(atom_batch, minlength=n_b)
    rc = np.bincount(residue_batch, minlength=n_b)
    a_off = np.concatenate([[0], np.cumsum(ac)])
    r_off = np.concatenate([[0], np.cumsum(rc)])

    G = (n_b + N_CORES - 1) // N_CORES
    AG = max(P, int(np.ceil(ac.max() / P)) * P)
    RG = max(P, int(np.ceil(rc.max() / P)) * P)
    A_pad, R_pad = G * AG, G * RG
    nkg = RG // P
    nRc = G * nkg

    key = (AG, RG, G)
    if key not in _kernel_cache:
        _kernel_cache[key] = _build_kernel(AG, RG, G)
    nc = _kernel_cache[key]

    wqT = np.ascontiguousarray(W_q.T)
    wkT = np.ascontiguousarray(W_k.T)
    wvT = np.ascontiguousarray(W_v.T)

    in_maps = []
    for c in range(N_CORES):
        atomT_c = np.zeros((P, A_pad), dtype=np.float32)
        resT_c = np.zeros((P, R_pad), dtype=np.float32)
        valid_c = np.zeros((P, nRc), dtype=np.float32)
        for j in range(G):
            g = c * G + j
            if g >= n_b:
                continue
            na, nr = int(ac[g]), int(rc[g])
            if na:
                atomT_c[:, j * AG : j * AG + na] = atom_h[a_off[g] : a_off[g] + na].T
            if nr:
                resT_c[:, j * RG : j * RG + nr] = residue_h[r_off[g] : r_off[g] + nr].T
                # valid flags, flattened (chunk, partition) order
                flat = np.zeros(RG, dtype=np.float32)
                flat[:nr] = 1.0
                valid_c[:, j * nkg : (j + 1) * nkg] = flat.reshape(nkg, P).T
        in_maps.append({
            "atomT": atomT_c, "resT": resT_c,
            "wqT": wqT, "wkT": wkT, "wvT": wvT,
            "valid": valid_c,
        })

    res = run_bass_kernel_spmd(nc, in_maps, core_ids=list(range(N_CORES)))

    result = atom_h.copy()
    for c in range(N_CORES):
        u = res.results[c]["out"]
        for j in range(G):
            g = c * G + j
            if g >= n_b:
                continue
            na, nr = int(ac[g]), int(rc[g])
            if na == 0 or nr == 0:
                continue
            rows = u[j * AG : j * AG + na]
            result[a_off[g] : a_off[g] + na] += rows[:, :DH] / rows[:, DH : DH + 1]
    return result


# revision 5
# speedup vs baseline: 1.5691x; 1.5691x over previous
"""Cross-graph attention (block-diagonal segment-local attention) on 8 trn2 cores.

Strategy: graphs (batch ids) are contiguous segments in the sorted
atom_batch / residue_batch arrays.  Attention is block-diagonal: atoms of
graph b attend only to residues of graph b.  We shard 4 graphs per core,
pad every graph to a fixed (AG atoms, RG residues) slot so all 8 cores run
one identical SPMD program, and compute per-graph attention with no masks:

  - inputs are packed host-side as transposed tiles atom_h^T (128, A_pad),
    residue_h^T (128, R_pad); zero padding makes padded K columns / V rows
    exactly 0.
  - scores are computed transposed,  S^T = K @ Q^T,  so every matmul takes
    naturally-laid-out operands (no on-device transposes anywhere).
  - all matmuls run in float32r (fast fp32 mode, 1 cycle/row at free>=256).
  - exp(S/sqrt(128) + bias) is one ACT instruction per tile; the per-partition
    bias is 0 for real residues and -30000 for padded ones, so padded
    residues contribute exp = 0 downstream (mask costs zero instructions).
  - V is augmented with a ones column; U = expS^T.T @ [V | 1 | pad] then
    yields both the unnormalized context and the softmax denominator.
  - normalization + residual add run host-side: out = atom_h + U[:, :128]/U[:, 128:129].
"""

import sys

if "/opt/trn_rl_repo" not in sys.path:
    sys.path.insert(0, "/opt/trn_rl_repo")

import numpy as np

import concourse.bass as bass
import concourse.tile as tile
from concourse import bacc, mybir
from concourse.bass_utils import run_bass_kernel_spmd

N_CORES = 8
B = 32                      # number of graphs
P = 128                     # partitions
DH = 128                    # feature dims (DA == DR == DH == 128)
VW = 256                    # U-matmul rhs width (>=256 keeps f32r at full rate)
SCALE = 1.0 / np.sqrt(128.0)
NEG_BIAS = -30000.0

_kernel_cache: dict = {}


def _col_chunks(n):
    """Split n columns into matmul chunks of <=512 that never cross a
    512-element PSUM bank boundary (matmul output must stay in one bank)."""
    out, i = [], 0
    while i < n:
        w = min(512, n - i)
        out.append((i, w))
        i += w
    return out


def _build_kernel(AG: int, RG: int, G: int):
    """One SPMD program: G graph slots of (AG atoms, RG residues) per core."""
    A_pad = G * AG
    R_pad = G * RG
    nkg = RG // P               # residue chunks per graph
    nRc = G * nkg               # residue chunks per core
    ntg = AG // P               # atom chunks per graph
    nAc = G * ntg               # atom chunks per core
    f32 = mybir.dt.float32
    f32r = mybir.dt.float32r

    nc = bacc.Bacc("TRN2")
    atomT = nc.dram_tensor("atomT", [P, A_pad], f32r, kind="ExternalInput")
    resT = nc.dram_tensor("resT", [P, R_pad], f32r, kind="ExternalInput")
    wqT = nc.dram_tensor("wqT", [P, DH], f32r, kind="ExternalInput")
    wkT = nc.dram_tensor("wkT", [P, DH], f32r, kind="ExternalInput")
    wvT = nc.dram_tensor("wvT", [P, DH], f32r, kind="ExternalInput")
    bias = nc.dram_tensor("bias", [P, nRc], f32, kind="ExternalInput")
    out = nc.dram_tensor("out", [A_pad, DH + 1], f32, kind="ExternalOutput")

    sg_chunks = _col_chunks(AG)

    with tile.TileContext(nc) as tc:
        with (
            tc.tile_pool(name="singles", bufs=1) as singles,
            tc.tile_pool(name="psum_big", bufs=2, space="PSUM") as ps_big,
            tc.tile_pool(name="psum_small", bufs=2, space="PSUM") as ps_small,
        ):
            # ---- load everything to SBUF ----
            atomT_sb = singles.tile([P, A_pad], f32r)
            resT_sb = singles.tile([P, R_pad], f32r)
            wqT_sb = singles.tile([P, DH], f32r)
            wkT_sb = singles.tile([P, DH], f32r)
            wvT_sb = singles.tile([P, VW], f32r)
            bias_sb = singles.tile([P, nRc], f32)
            nc.sync.dma_start(atomT_sb[:], atomT[:])
            nc.sync.dma_start(resT_sb[:], resT[:])
            nc.sync.dma_start(wqT_sb[:], wqT[:])
            nc.sync.dma_start(wkT_sb[:], wkT[:])
            nc.vector.memset(wvT_sb[:].bitcast(f32), 0.0)
            nc.sync.dma_start(wvT_sb[:, :DH], wvT[:])
            nc.sync.dma_start(bias_sb[:], bias[:])

            # V' = [residue_h @ W_v^T | 1 | junk] laid out per residue chunk
            V_sb = singles.tile([P, nRc, VW], f32r)
            nc.vector.memset(V_sb[:].bitcast(f32), 1.0)

            # ---- Q^T = W_q @ atom_h^T, K^T = W_k @ residue_h^T ----
            QT_sb = singles.tile([P, A_pad], f32r)
            for i in range(0, A_pad, 512):
                w = min(512, A_pad - i)
                pq = ps_big.tile([P, 512], f32, tag="big")
                nc.tensor.matmul(
                    pq[:, :w], wqT_sb[:], atomT_sb[:, i : i + w],
                    start=True, stop=True,
                )
                nc.vector.tensor_copy(QT_sb[:, i : i + w], pq[:, :w])

            KT_sb = singles.tile([P, R_pad], f32r)
            for i in range(0, R_pad, 512):
                w = min(512, R_pad - i)
                pk = ps_big.tile([P, 512], f32, tag="big")
                nc.tensor.matmul(
                    pk[:, :w], wkT_sb[:], resT_sb[:, i : i + w],
                    start=True, stop=True,
                )
                nc.vector.tensor_copy(KT_sb[:, i : i + w], pk[:, :w])

            # ---- V chunks (rhs padded to VW cols so f32r runs at rate 1) ----
            for k in range(nRc):
                pv = ps_small.tile([P, VW], f32, tag="small")
                nc.tensor.matmul(
                    pv[:], resT_sb[:, k * P : (k + 1) * P], wvT_sb[:],
                    start=True, stop=True,
                )
                nc.vector.tensor_copy(V_sb[:, k, :DH], pv[:, :DH])

            # ---- per-graph attention ----
            ES_sb = singles.tile([P, nRc, AG], f32r)   # exp(S^T) per residue chunk
            OUT_sb = singles.tile([P, nAc, DH + 1], f32)

            for g in range(G):
                a0 = g * AG
                for k in range(nkg):
                    kg = g * nkg + k
                    r0 = kg * P
                    ps = ps_big.tile([P, 512 * ((AG + 511) // 512)], f32, tag="big")
                    for c, w in sg_chunks:
                        nc.tensor.matmul(
                            ps[:, c : c + w],
                            KT_sb[:, r0 : r0 + P],
                            QT_sb[:, a0 + c : a0 + c + w],
                            start=True, stop=True,
                        )
                    nc.scalar.activation(
                        ES_sb[:, kg, :], ps[:, :AG],
                        mybir.ActivationFunctionType.Exp,
                        bias=bias_sb[:, kg : kg + 1], scale=SCALE,
                    )

                for t in range(ntg):
                    tg = g * ntg + t
                    pu = ps_small.tile([P, VW], f32, tag="small")
                    for k in range(nkg):
                        kg = g * nkg + k
                        nc.tensor.matmul(
                            pu[:],
                            ES_sb[:, kg, t * P : (t + 1) * P],
                            V_sb[:, kg, :],
                            start=(k == 0), stop=(k == nkg - 1),
                        )
                    nc.vector.tensor_copy(OUT_sb[:, tg, :], pu[:, : DH + 1])

            nc.sync.dma_start(
                out.rearrange("(t p) f -> p t f", p=P), OUT_sb[:]
            )

    nc.compile()
    return nc


def kernel(atom_h, residue_h, atom_batch, residue_batch, W_q, W_k, W_v):
    atom_h = np.asarray(atom_h, dtype=np.float32)
    residue_h = np.asarray(residue_h, dtype=np.float32)
    atom_batch = np.asarray(atom_batch)
    residue_batch = np.asarray(residue_batch)
    W_q = np.asarray(W_q, dtype=np.float32)
    W_k = np.asarray(W_k, dtype=np.float32)
    W_v = np.asarray(W_v, dtype=np.float32)

    A = atom_h.shape[0]
    R = residue_h.shape[0]
    n_b = max(B, int(atom_batch.max()) + 1 if A else B,
              int(residue_batch.max()) + 1 if R else B)

    ac = np.bincount(atom_batch, minlength=n_b)
    rc = np.bincount(residue_batch, minlength=n_b)
    a_off = np.concatenate([[0], np.cumsum(ac)])
    r_off = np.concatenate([[0], np.cumsum(rc)])

    G = (n_b + N_CORES - 1) // N_CORES
    AG = max(P, int(np.ceil(ac.max() / P)) * P)
    RG = max(P, int(np.ceil(rc.max() / P)) * P)
    A_pad, R_pad = G * AG, G * RG
    nkg = RG // P
    nRc = G * nkg

    key = (AG, RG, G)
    if key not in _kernel_cache:
        _kernel_cache[key] = _build_kernel(AG, RG, G)
    nc = _kernel_cache[key]

    wqT = np.ascontiguousarray(W_q.T)
    wkT = np.ascontiguousarray(W_k.T)
    wvT = np.ascontiguousarray(W_v.T)

    in_maps = []
    for c in range(N_CORES):
        atomT_c = np.zeros((P, A_pad), dtype=np.float32)
        resT_c = np.zeros((P, R_pad), dtype=np.float32)
        bias_c = np.zeros((P, nRc), dtype=np.float32)
        for j in range(G):
            g = c * G + j
            if g >= n_b:
                bias_c[:, j * nkg : (j + 1) * nkg] = NEG_BIAS
                continue
            na, nr = int(ac[g]), int(rc[g])
            if na:
                atomT_c[:, j * AG : j * AG + na] = atom_h[a_off[g] : a_off[g] + na].T
            if nr:
                resT_c[:, j * RG : j * RG + nr] = residue_h[r_off[g] : r_off[g] + nr].T
            flat = np.full(RG, NEG_BIAS, dtype=np.float32)
            flat[:nr] = 0.0
            bias_c[:, j * nkg : (j + 1) * nkg] = flat.reshape(nkg, P).T
        in_maps.append({
            "atomT": atomT_c, "resT": resT_c,
            "wqT": wqT, "wkT": wkT, "wvT": wvT,
            "bias": bias_c,
        })

    res = run_bass_kernel_spmd(nc, in_maps, core_ids=list(range(N_CORES)))

    result = atom_h.copy()
    for c in range(N_CORES):
        u = res.results[c]["out"]
        for j in range(G):
            g = c * G + j
            if g >= n_b:
                continue
            na, nr = int(ac[g]), int(rc[g])
            if na == 0 or nr == 0:
                continue
            rows = u[j * AG : j * AG + na]
            result[a_off[g] : a_off[g] + na] += rows[:, :DH] / rows[:, DH : DH + 1]
    return result
